# revision 43
# baseline (speedup 1.0000x reference)
"""Multi-head cross-attention (B=4, S=2048, D=1024, H=16) on 8 Trainium2 cores.

Sharding: hybrid data/tensor parallel. Core c handles batch b = c//2 and
head-group g = c%2 (8 of the 16 heads, i.e. 512 of the 1024 q/k/v dims).
Each core computes a partial out-projection over its 512 attention dims;
the host sums the two partials per batch (the tensor-parallel all-reduce
after out_lin, done on host since pairs share a batch).

Per-core kernel (all matmuls f16, f32 PSUM):
  K.T = wk_t.T @ mem_t (+bk)                  [512, 2048]
  V   = mem_t.T @ wv_t -> v_aug [*, 8, 65]    (ones column per head = denom)
  Q.T = (wq_t.T @ x_t + bq) * 184.664         [512, 2048]
        (scale = 0.125*log2(e)*1024: logits arrive in 1024ths of an octave)
  attention runs as a flat stream of (head, query-half, key-chunk) units,
  software-pipelined 2 deep ACROSS block boundaries (the PE executes its
  queue in order, so the last AV of a block must not wait on a fresh exp):
    S.T[k,q] = K_h @ Q_h.T                    (PSUM f32)
    P.T      = 2^(S.T/1024 - 1/2), alternating between TWO engines:
               ACT: native Exp(scale=ln2/1024, bias=-ln2/2)
               DVE: custom 8-stage op EXP2_OCTAVE_ANT (round-to-octave via
               f32 magic-add, quadratic mantissa, i16 writeback = f16 bits;
               max rel err 3e-3). The -1/2 octave shift cancels in softmax.
    AV.T    += [V_h|1].T @ P.T                [65, 1024] PSUM accum over k
    denom: ACT copies AV.T[64] -> SBUF (ACT can cross partitions), DVE
           reciprocal_approx_fast, GpSimd partition_broadcast, DVE multiply
  out.T = wo_t.T @ attn.T (+bo_eff on g==0)   [1024, 2048] partial, f16,
  emitted per query-half so evac/DMA overlap the next half's attention.

bv is folded into bo on the host (softmax rows sum to 1). Key-padding mask
(all-False in this problem) uses a lazily-built ACT-only exp variant.
"""

import numpy as np

import concourse.bacc as bacc
import concourse.mybir as mybir
from concourse import tile
from concourse.bass_utils import run_bass_kernel_spmd

F32 = mybir.dt.float32
F16 = mybir.dt.float16
I16 = mybir.dt.int16
AF = mybir.ActivationFunctionType
ALU = mybir.AluOpType

B, S, D = 4, 2048, 1024
H, HD = 16, 64
NCORES = 8
NH = 8          # heads per core
OD = NH * HD    # 512 attention dims per core
P = 128
NDC = D // P    # 8 d-chunks
NKC = S // P    # 16 key chunks

LOG2E = float(np.log2(np.e))
LN2 = float(np.log(2.0))
SCALE_Q = 0.125 * LOG2E * 1024.0          # 184.664: q prescale
ACT_SCALE = LN2 / 1024.0                  # ACT exp: 2^(lg/1024) * 2^-0.5
ACT_BIAS = -0.5 * LN2
MASK_OCT = -40.0 * 1024.0                 # masked key: 2^-40 ~ 0

# custom DVE exp2-octave constants (derivation in work/probe2.py)
EXP_C2 = 3.31802762e-04
EXP_K = 1.5 * 2**33
EXP_IMM2 = 4.25019897e+02 - 1024.0 + 15360.0

ACT_KC = frozenset({0, 1, 2, 4, 6, 8, 10, 12, 14})   # 9/16 chunks on ACT

_cache = {}


def _register_exp_op():
    import concourse.dve_ops as dve_ops
    from concourse.dve_spec import Spec, Src0, C0, C1, C2, One, lower
    from concourse.dve_uop import DveOpSpec

    name = "EXP2_OCTAVE_ANT"
    for op in dve_ops.OPS:
        if op.name == name:
            return op

    t = Src0 + C0
    i2 = t - C0
    r = Src0 - i2
    q2 = (r * C1) + One
    body = (((q2 * r) + C2) + i2)

    def ref(in0, in1, s0, s1, imm2):
        tt = (in0.astype(np.float32) + np.float32(s0)).astype(np.float32)
        ii = (tt - np.float32(s0)).astype(np.float32)
        rr = (in0.astype(np.float32) - ii).astype(np.float32)
        return ((rr * np.float32(s1) + np.float32(1.0)) * rr
                + np.float32(imm2)) + ii

    spec = Spec(body=body, reference=ref)
    shas = {}
    for ver in ("v3", "v4"):
        try:
            s = DveOpSpec(name=name, opcode=1, uops=lower(spec, ver=ver),
                          rd1_en=False)
            shas[ver] = s.sha(ver)
        except Exception:
            pass
    op = dve_ops.DveOp(name, spec, subdim=False, uops_sha=shas)
    dve_ops.OPS.append(op)
    dve_ops.CUSTOM_DVE_SPECS[name] = spec
    dve_ops._SUB_OPCODE_FOR_NAME[name] = (
        dve_ops._CUSTOM_DVE_ROW_BASE + len(dve_ops.OPS) - 1)
    return op


def _build(masked: bool):
    from contextlib import ExitStack

    exp_op = _register_exp_op()
    nc = bacc.Bacc(None, target_bir_lowering=False, debug=False)

    x_t = nc.dram_tensor("x_t", [D, S], F16, kind="ExternalInput").ap()
    mem_t = nc.dram_tensor("mem_t", [D, S], F16, kind="ExternalInput").ap()
    wq_t = nc.dram_tensor("wq_t", [D, OD], F16, kind="ExternalInput").ap()
    wk_t = nc.dram_tensor("wk_t", [D, OD], F16, kind="ExternalInput").ap()
    wv_t = nc.dram_tensor("wv_t", [D, OD], F16, kind="ExternalInput").ap()
    wo_t = nc.dram_tensor("wo_t", [OD, D], F16, kind="ExternalInput").ap()
    consts = nc.dram_tensor("consts", [P, 32], F32, kind="ExternalInput").ap()
    out_t = nc.dram_tensor("out_t", [D, S], F16, kind="ExternalOutput").ap()

    x_c = x_t.rearrange("(c i p) s -> c p i s", p=P, i=2)
    m_c = mem_t.rearrange("(c i p) s -> c p i s", p=P, i=2)
    wq_r = wq_t.rearrange("(i p) o -> p i o", p=P)
    wk_r = wk_t.rearrange("(i p) o -> p i o", p=P)
    wv_r = wv_t.rearrange("(i p) o -> p i o", p=P)
    wo_r = wo_t.rearrange("(i p) o -> p i o", p=P)

    dq = [nc.sync, nc.gpsimd, nc.scalar]

    with tile.TileContext(nc) as tc, ExitStack() as ctx:
        q_pool = ctx.enter_context(tc.tile_pool(name="qt", bufs=1))
        k_pool = ctx.enter_context(tc.tile_pool(name="kt", bufs=1))
        v_pool = ctx.enter_context(tc.tile_pool(name="va", bufs=1))
        a_pool = ctx.enter_context(tc.tile_pool(name="at", bufs=1))
        c_pool = ctx.enter_context(tc.tile_pool(name="cst", bufs=1))
        w_pool = ctx.enter_context(tc.tile_pool(name="wt", bufs=1))
        e_pool = ctx.enter_context(tc.tile_pool(name="es", bufs=4))
        d_pool = ctx.enter_context(tc.tile_pool(name="dn", bufs=2))
        o_pool = ctx.enter_context(tc.tile_pool(name="ev", bufs=5))
        psum_pool = ctx.enter_context(tc.tile_pool(name="ps", bufs=2,
                                                   space="PSUM"))
        xm_pool = ctx.enter_context(tc.tile_pool(name="xm", bufs=1))

        # ---- consolidated loads: few big DMAs, spread over 3 queues.
        #      wk + mem first (K proj's critical path), wk halves on two
        #      queues; x loads overlap compute via slot reuse:
        #      x0 -> wk's slot (free after K proj), x1 -> its own tile
        #      (transfers during V proj), x2/x3 -> m0/m1 slots ----
        wk_sb = w_pool.tile([P, NDC, OD], F16, tag="wk", name="wk")
        nc.sync.dma_start(out=wk_sb[:, 0:4, :], in_=wk_r[:, 0:4, :])
        nc.scalar.dma_start(out=wk_sb[:, 4:8, :], in_=wk_r[:, 4:8, :])
        m_til = []
        for c in range(4):
            t = xm_pool.tile([P, 2, S], F16, tag=f"m{c}", name=f"m{c}")
            [nc.gpsimd, nc.sync, nc.scalar, nc.gpsimd][c].dma_start(
                out=t[:], in_=m_c[c])
            m_til.append(t)
        cst = c_pool.tile([P, 32], F32, tag="cst")
        nc.sync.dma_start(out=cst[:], in_=consts[:])
        bq_sb, bk_sb = cst[:, 0:4], cst[:, 4:8]
        bo_sb, mk_sb = cst[:, 8:16], cst[:, 16:32]
        wv_sb = w_pool.tile([P, NDC, OD], F16, tag="wv", name="wv")
        nc.gpsimd.dma_start(out=wv_sb[:], in_=wv_r[:])
        wq_sb = w_pool.tile([P, NDC, OD], F16, tag="wq", name="wq")
        nc.scalar.dma_start(out=wq_sb[:], in_=wq_r[:])
        wo_sb = w_pool.tile([P, OD // P, D], F16, tag="wo", name="wo")
        nc.gpsimd.dma_start(out=wo_sb[:], in_=wo_r[:])
        # x1 has its own tile: transfers while V proj computes
        x1_t = xm_pool.tile([P, 2, S], F16, tag="x1", name="x1")
        nc.scalar.dma_start(out=x1_t[:], in_=x_c[1])

        # ---- persistent tiles ----
        qT = [q_pool.tile([P, S], F16, tag=f"q{m}", name=f"q{m}")
              for m in range(OD // P)]
        kT = [k_pool.tile([P, S], F16, tag=f"k{h}", name=f"k{h}")
              for h in range(NH)]
        for h in range(NH):
            ro = 64 * (h % 2)
            nc.vector.memset(kT[h][64 - ro:128 - ro, :], 0.0)
        v_aug = [v_pool.tile([P, 9, 65], F16, tag=f"v{st}", name=f"v{st}")
                 for st in range(NKC)]
        ones_f = c_pool.tile([P, NH], F32, tag="onef")
        nc.gpsimd.memset(ones_f[:], 1.0)
        ones_r = c_pool.tile([P, NH], F16, tag="oner")
        nc.gpsimd.tensor_copy(ones_r[:], ones_f[:])
        for st in range(NKC):
            nc.gpsimd.memset(v_aug[st][:, 8, :], 0.0)
            nc.gpsimd.tensor_copy(v_aug[st][:, 0:NH, 64:65],
                                  ones_r[:].unsqueeze(2))
        attn = [a_pool.tile([P, S], F16, tag=f"a{m}", name=f"a{m}")
                for m in range(OD // P)]

        def m_sl(i, csl):
            return m_til[i // 2][:, i % 2, csl]

        # ---- K.T projection (PE starts on wk's first half as it lands) ----
        for m in range(OD // P):
            msl = slice(m * P, (m + 1) * P)
            for n in range(2):
                csl = slice(n * 1024, (n + 1) * 1024)
                ps = psum_pool.tile([P, 1024], F32, tag="lg", name="psk")
                for i in range(NDC):
                    for j in range(2):
                        nc.tensor.matmul(
                            ps[:, j * 512:(j + 1) * 512],
                            wk_sb[:, i, msl],
                            m_sl(i, slice(n * 1024 + j * 512,
                                          n * 1024 + (j + 1) * 512)),
                            start=(i == 0), stop=(i == NDC - 1),
                        )
                nc.scalar.activation(
                    kT[2 * m][0:64, csl], ps[0:64, :], AF.Identity,
                    bias=bk_sb[0:64, m:m + 1])
                nc.scalar.activation(
                    kT[2 * m + 1][64:128, csl], ps[64:128, :], AF.Identity,
                    bias=bk_sb[64:128, m:m + 1])

        # ---- V into v_aug ----
        for st in range(NKC):
            ps = psum_pool.tile([P, 1024], F32, tag="lg", name="psv")
            for i in range(NDC):
                nc.tensor.matmul(
                    ps[:, 0:OD], m_sl(i, slice(st * P, (st + 1) * P)),
                    wv_sb[:, i, :],
                    start=(i == 0), stop=(i == NDC - 1),
                )
            nc.vector.tensor_copy(
                v_aug[st][:, 0:NH, 0:64],
                ps[:, 0:OD].rearrange("p (h d) -> p h d", h=NH),
            )

        # ---- x loads: x0 -> wk slot, x2/x3 -> m0/m1 slots ----
        x0_t = w_pool.tile([P, 2, S], F16, tag="wk", name="x0")
        nc.sync.dma_start(out=x0_t[:], in_=x_c[0])
        x2_t = xm_pool.tile([P, 2, S], F16, tag="m0", name="x2")
        nc.sync.dma_start(out=x2_t[:], in_=x_c[2])
        x3_t = xm_pool.tile([P, 2, S], F16, tag="m1", name="x3")
        nc.gpsimd.dma_start(out=x3_t[:], in_=x_c[3])
        x_til = [x0_t, x1_t, x2_t, x3_t]

        def x_sl(i, csl):
            return x_til[i // 2][:, i % 2, csl]

        # ---- Q.T projections (prescaled by SCALE_Q) ----
        for mt in range(OD // P):
            msl = slice(mt * P, (mt + 1) * P)
            for n in range(2):
                csl = slice(n * 1024, (n + 1) * 1024)
                ps = psum_pool.tile([P, 1024], F32, tag="lg", name="psq")
                for i in range(NDC):
                    for j in range(2):
                        nc.tensor.matmul(
                            ps[:, j * 512:(j + 1) * 512],
                            wq_sb[:, i, msl],
                            x_sl(i, slice(n * 1024 + j * 512,
                                          n * 1024 + (j + 1) * 512)),
                            start=(i == 0), stop=(i == NDC - 1),
                        )
                nc.scalar.activation(
                    qT[mt][:, csl], ps[:], AF.Identity,
                    bias=bq_sb[:, mt:mt + 1], scale=SCALE_Q)

        # ---- attention: flat (block, kc) unit stream, pipelined 2 deep
        #      across block boundaries; out-proj after each query-half ----
        blocks = [(mt, 2 * mt + hh, qh)
                  for qh in range(2) for mt in range(4) for hh in range(2)]
        units = [(bi, kc) for bi in range(len(blocks)) for kc in range(NKC)]

        def emit_lg(u):
            bi, kc = units[u]
            mt, h, qh = blocks[bi]
            lg = psum_pool.tile([P, 1024], F32, tag="lg", name="lg")
            for j in range(2):
                nc.tensor.matmul(
                    lg[:, j * 512:(j + 1) * 512],
                    kT[h][:, kc * P:(kc + 1) * P],
                    qT[mt][:, qh * 1024 + j * 512:
                            qh * 1024 + (j + 1) * 512],
                    start=True, stop=True,
                )
            return lg

        def emit_es(u, lg):
            bi, kc = units[u]
            if masked or kc in ACT_KC:
                es = e_pool.tile([P, 1024], F16, tag="es")
                nc.scalar.activation(
                    es[:], lg[:], AF.Exp,
                    bias=mk_sb[:, kc:kc + 1], scale=ACT_SCALE)
                return es[:]
            es = e_pool.tile([P, 1024], I16, tag="es")
            nc.vector._custom_dve(exp_op, out=es[:], in0=lg[:],
                                  s0=EXP_K, s1=EXP_C2, imm2=EXP_IMM2)
            return es[:].bitcast(F16)

        def emit_out_proj(m, qh):
            csl = slice(qh * 1024, (qh + 1) * 1024)
            ps = psum_pool.tile([P, 1024], F32, tag="av", name="pso")
            for i in range(OD // P):
                for j in range(2):
                    nc.tensor.matmul(
                        ps[:, j * 512:(j + 1) * 512],
                        wo_sb[:, i, m * P:(m + 1) * P],
                        attn[i][:, qh * 1024 + j * 512:
                                qh * 1024 + (j + 1) * 512],
                        start=(i == 0), stop=(i == OD // P - 1),
                    )
            ev = o_pool.tile([P, 1024], F16, tag="ev")
            if m % 2 == 0:
                nc.vector.tensor_scalar_add(ev[:], ps[:], bo_sb[:, m:m + 1])
            else:
                nc.scalar.activation(
                    ev[:], ps[:], AF.Identity, bias=bo_sb[:, m:m + 1])
            dq[m % 3].dma_start(out=out_t[m * P:(m + 1) * P, csl], in_=ev[:])

        lg_q = [emit_lg(0), emit_lg(1)]
        av_t = None
        def emit_chain(av, mt, h, qh):
            # softmax denominator: copy (ACT crosses partitions) ->
            # recip -> bcast -> mult
            ro = 64 * (h % 2)
            den = d_pool.tile([1, 1024], F32, tag="den")
            nc.scalar.activation(den[:], av[64:65, :], AF.Copy)
            rcp = d_pool.tile([1, 1024], F32, tag="rcp")
            nc.vector.reciprocal_approx_fast(rcp[:], den[:])
            bc = d_pool.tile([64, 1024], F32, tag="bc")
            nc.gpsimd.partition_broadcast(bc[:], rcp[:])
            nc.vector.tensor_mul(
                attn[mt][ro:ro + 64, qh * 1024:(qh + 1) * 1024],
                av[0:64, :], bc[:])

        for u in range(len(units)):
            bi, kc = units[u]
            mt, h, qh = blocks[bi]
            if kc == 0:
                av_t = psum_pool.tile([P, 1024], F32, tag="av", name="av")
            es_mm = emit_es(u, lg_q.pop(0))
            if u + 2 < len(units):
                lg_q.append(emit_lg(u + 2))
            va_flat = v_aug[kc][:].rearrange("p h d -> p (h d)")
            for j in range(2):
                nc.tensor.matmul(
                    av_t[:, j * 512:(j + 1) * 512],
                    va_flat[:, 65 * h:65 * h + 128],
                    es_mm[:, j * 512:(j + 1) * 512],
                    start=(kc == 0), stop=(kc == NKC - 1),
                )
            if kc == NKC - 1:
                emit_chain(av_t, mt, h, qh)
                if bi % 8 == 7:          # last block of this query-half
                    for m in range(D // P):
                        emit_out_proj(m, qh)

    nc.compile()
    return nc


def _prep_inputs(x, memory, mask, wq, bq, wk, bk, wv, bv, wo, bo):
    f = np.float32
    h = np.float16
    wqT = np.ascontiguousarray(wq.T, dtype=f)
    wkT = np.ascontiguousarray(wk.T, dtype=f)
    wvT = np.ascontiguousarray(wv.T, dtype=f)
    woT = np.ascontiguousarray(wo.T, dtype=f)
    bo_eff = (bo.astype(f) + wo.astype(f) @ bv.astype(f))
    zeros_bo = np.zeros_like(bo_eff)
    in_maps = []
    for c in range(NCORES):
        b, g = divmod(c, 2)
        sl = slice(g * OD, (g + 1) * OD)
        bo_c = bo_eff if g == 0 else zeros_bo
        cst = np.empty((P, 32), f)
        cst[:, 0:4] = bq[sl].astype(f).reshape(OD // P, P).T
        cst[:, 4:8] = bk[sl].astype(f).reshape(OD // P, P).T
        cst[:, 8:16] = bo_c.reshape(D // P, P).T
        cst[:, 16:32] = (
            np.where(mask[b], np.float32(MASK_OCT * ACT_SCALE), 0.0)
            + np.float32(ACT_BIAS)).astype(f).reshape(NKC, P).T
        in_maps.append({
            "x_t": np.ascontiguousarray(x[b].T, dtype=h),
            "mem_t": np.ascontiguousarray(memory[b].T, dtype=h),
            "wq_t": np.ascontiguousarray(wqT[:, sl]).astype(h),
            "wk_t": np.ascontiguousarray(wkT[:, sl]).astype(h),
            "wv_t": np.ascontiguousarray(wvT[:, sl]).astype(h),
            "wo_t": np.ascontiguousarray(woT[sl, :]).astype(h),
            "consts": cst,
        })
    return in_maps


def kernel(x, memory, mask, wq, bq, wk, bk, wv, bv, wo, bo, **run_kwargs):
    x = np.asarray(x, dtype=np.float32)
    memory = np.asarray(memory, dtype=np.float32)
    mask = np.asarray(mask)
    masked = bool(mask.any())
    key = "nc_masked" if masked else "nc"
    if key not in _cache:
        _cache[key] = _build(masked)
    nc = _cache[key]
    in_maps = _prep_inputs(x, memory, mask, wq, bq, wk, bk, wv, bv, wo, bo)
    res = run_bass_kernel_spmd(nc, in_maps, list(range(NCORES)), **run_kwargs)
    out = np.empty((B, S, D), dtype=np.float32)
    for b in range(B):
        part = (res.results[2 * b]["out_t"].astype(np.float32)
                + res.results[2 * b + 1]["out_t"].astype(np.float32))
        out[b] = part.T
    if run_kwargs:
        _cache["last_results"] = res
    return out


# revision 44
# speedup vs baseline: 1.0034x; 1.0034x over previous
"""Multi-head cross-attention (B=4, S=2048, D=1024, H=16) on 8 Trainium2 cores.

Sharding: hybrid data/tensor parallel. Core c handles batch b = c//2 and
head-group g = c%2 (8 of the 16 heads, i.e. 512 of the 1024 q/k/v dims).
Each core computes a partial out-projection over its 512 attention dims;
the host sums the two partials per batch (the tensor-parallel all-reduce
after out_lin, done on host since pairs share a batch).

Per-core kernel (all matmuls f16, f32 PSUM):
  K.T = wk_t.T @ mem_t (+bk)                  [512, 2048]
  V   = mem_t.T @ wv_t -> v_aug [*, 8, 65]    (ones column per head = denom)
  Q.T = (wq_t.T @ x_t + bq) * 184.664         [512, 2048]
        (scale = 0.125*log2(e)*1024: logits arrive in 1024ths of an octave)
  attention runs as a flat stream of (head, query-half, key-chunk) units,
  software-pipelined 2 deep ACROSS block boundaries (the PE executes its
  queue in order, so the last AV of a block must not wait on a fresh exp):
    S.T[k,q] = K_h @ Q_h.T                    (PSUM f32)
    P.T      = 2^(S.T/1024 - 1/2), alternating between TWO engines:
               ACT: native Exp(scale=ln2/1024, bias=-ln2/2)
               DVE: custom 8-stage op EXP2_OCTAVE_ANT (round-to-octave via
               f32 magic-add, quadratic mantissa, i16 writeback = f16 bits;
               max rel err 3e-3). The -1/2 octave shift cancels in softmax.
    AV.T    += [V_h|1].T @ P.T                [65, 1024] PSUM accum over k
    denom: ACT copies AV.T[64] -> SBUF (ACT can cross partitions), DVE
           reciprocal_approx_fast, GpSimd partition_broadcast, DVE multiply
  out.T = wo_t.T @ attn.T (+bo_eff on g==0)   [1024, 2048] partial, f16,
  emitted per query-half so evac/DMA overlap the next half's attention.

bv is folded into bo on the host (softmax rows sum to 1). Key-padding mask
(all-False in this problem) uses a lazily-built ACT-only exp variant.
"""

import numpy as np

import concourse.bacc as bacc
import concourse.mybir as mybir
from concourse import tile
from concourse.bass_utils import run_bass_kernel_spmd

F32 = mybir.dt.float32
F16 = mybir.dt.float16
I16 = mybir.dt.int16
AF = mybir.ActivationFunctionType
ALU = mybir.AluOpType

B, S, D = 4, 2048, 1024
H, HD = 16, 64
NCORES = 8
NH = 8          # heads per core
OD = NH * HD    # 512 attention dims per core
P = 128
NDC = D // P    # 8 d-chunks
NKC = S // P    # 16 key chunks

LOG2E = float(np.log2(np.e))
LN2 = float(np.log(2.0))
SCALE_Q = 0.125 * LOG2E * 1024.0          # 184.664: q prescale
ACT_SCALE = LN2 / 1024.0                  # ACT exp: 2^(lg/1024) * 2^-0.5
ACT_BIAS = -0.5 * LN2
MASK_OCT = -40.0 * 1024.0                 # masked key: 2^-40 ~ 0

# custom DVE exp2-octave constants (derivation in work/probe2.py)
EXP_C2 = 3.31802762e-04
EXP_K = 1.5 * 2**33
EXP_IMM2 = 4.25019897e+02 - 1024.0 + 15360.0

ACT_KC = frozenset({0, 1, 2, 4, 6, 8, 10, 12, 14})   # 9/16 chunks on ACT

_cache = {}


def _register_exp_op():
    import concourse.dve_ops as dve_ops
    from concourse.dve_spec import Spec, Src0, C0, C1, C2, One, lower
    from concourse.dve_uop import DveOpSpec

    name = "EXP2_OCTAVE_ANT"
    for op in dve_ops.OPS:
        if op.name == name:
            return op

    t = Src0 + C0
    i2 = t - C0
    r = Src0 - i2
    q2 = (r * C1) + One
    body = (((q2 * r) + C2) + i2)

    def ref(in0, in1, s0, s1, imm2):
        tt = (in0.astype(np.float32) + np.float32(s0)).astype(np.float32)
        ii = (tt - np.float32(s0)).astype(np.float32)
        rr = (in0.astype(np.float32) - ii).astype(np.float32)
        return ((rr * np.float32(s1) + np.float32(1.0)) * rr
                + np.float32(imm2)) + ii

    spec = Spec(body=body, reference=ref)
    shas = {}
    for ver in ("v3", "v4"):
        try:
            s = DveOpSpec(name=name, opcode=1, uops=lower(spec, ver=ver),
                          rd1_en=False)
            shas[ver] = s.sha(ver)
        except Exception:
            pass
    op = dve_ops.DveOp(name, spec, subdim=False, uops_sha=shas)
    dve_ops.OPS.append(op)
    dve_ops.CUSTOM_DVE_SPECS[name] = spec
    dve_ops._SUB_OPCODE_FOR_NAME[name] = (
        dve_ops._CUSTOM_DVE_ROW_BASE + len(dve_ops.OPS) - 1)
    return op


def _build(masked: bool):
    from contextlib import ExitStack

    exp_op = _register_exp_op()
    nc = bacc.Bacc(None, target_bir_lowering=False, debug=False)

    x_t = nc.dram_tensor("x_t", [D, S], F16, kind="ExternalInput").ap()
    mem_t = nc.dram_tensor("mem_t", [D, S], F16, kind="ExternalInput").ap()
    wq_t = nc.dram_tensor("wq_t", [D, OD], F16, kind="ExternalInput").ap()
    wk_t = nc.dram_tensor("wk_t", [D, OD], F16, kind="ExternalInput").ap()
    wv_t = nc.dram_tensor("wv_t", [D, OD], F16, kind="ExternalInput").ap()
    wo_t = nc.dram_tensor("wo_t", [OD, D], F16, kind="ExternalInput").ap()
    consts = nc.dram_tensor("consts", [P, 32], F32, kind="ExternalInput").ap()
    out_t = nc.dram_tensor("out_t", [D, S], F16, kind="ExternalOutput").ap()

    x_c = x_t.rearrange("(c i p) s -> c p i s", p=P, i=2)
    m_c = mem_t.rearrange("(c i p) s -> c p i s", p=P, i=2)
    wq_r = wq_t.rearrange("(i p) o -> p i o", p=P)
    wk_r = wk_t.rearrange("(i p) o -> p i o", p=P)
    wv_r = wv_t.rearrange("(i p) o -> p i o", p=P)
    wo_r = wo_t.rearrange("(i p) o -> p i o", p=P)

    dq = [nc.sync, nc.gpsimd, nc.scalar]

    with tile.TileContext(nc) as tc, ExitStack() as ctx:
        q_pool = ctx.enter_context(tc.tile_pool(name="qt", bufs=1))
        k_pool = ctx.enter_context(tc.tile_pool(name="kt", bufs=1))
        v_pool = ctx.enter_context(tc.tile_pool(name="va", bufs=1))
        a_pool = ctx.enter_context(tc.tile_pool(name="at", bufs=1))
        c_pool = ctx.enter_context(tc.tile_pool(name="cst", bufs=1))
        w_pool = ctx.enter_context(tc.tile_pool(name="wt", bufs=1))
        e_pool = ctx.enter_context(tc.tile_pool(name="es", bufs=4))
        d_pool = ctx.enter_context(tc.tile_pool(name="dn", bufs=2))
        o_pool = ctx.enter_context(tc.tile_pool(name="ev", bufs=4))
        psum_pool = ctx.enter_context(tc.tile_pool(name="ps", bufs=2,
                                                   space="PSUM"))
        xm_pool = ctx.enter_context(tc.tile_pool(name="xm", bufs=1))

        # ---- consolidated loads: few big DMAs, spread over 3 queues.
        #      wk + mem first (K proj's critical path), wk halves on two
        #      queues; x loads overlap compute via slot reuse:
        #      x0 -> wk's slot (free after K proj), x1 -> its own tile
        #      (transfers during V proj), x2/x3 -> m0/m1 slots ----
        wk_sb = w_pool.tile([P, NDC, OD], F16, tag="wk", name="wk")
        nc.sync.dma_start(out=wk_sb[:, 0:4, :], in_=wk_r[:, 0:4, :])
        nc.scalar.dma_start(out=wk_sb[:, 4:8, :], in_=wk_r[:, 4:8, :])
        m_til = []
        for c in range(4):
            t = xm_pool.tile([P, 2, S], F16, tag=f"m{c}", name=f"m{c}")
            [nc.gpsimd, nc.sync, nc.scalar, nc.gpsimd][c].dma_start(
                out=t[:], in_=m_c[c])
            m_til.append(t)
        cst = c_pool.tile([P, 32], F32, tag="cst")
        nc.sync.dma_start(out=cst[:], in_=consts[:])
        bq_sb, bk_sb = cst[:, 0:4], cst[:, 4:8]
        bo_sb, mk_sb = cst[:, 8:16], cst[:, 16:32]
        wv_sb = w_pool.tile([P, NDC, OD], F16, tag="wv", name="wv")
        nc.gpsimd.dma_start(out=wv_sb[:], in_=wv_r[:])
        wq_sb = w_pool.tile([P, NDC, OD], F16, tag="wq", name="wq")
        nc.scalar.dma_start(out=wq_sb[:], in_=wq_r[:])
        wo_sb = w_pool.tile([P, OD // P, D], F16, tag="wo", name="wo")
        nc.gpsimd.dma_start(out=wo_sb[:], in_=wo_r[:])
        # x1 has its own tile: transfers while V proj computes
        x1_t = xm_pool.tile([P, 2, S], F16, tag="x1", name="x1")
        nc.scalar.dma_start(out=x1_t[:], in_=x_c[1])

        # ---- persistent tiles ----
        qT = [q_pool.tile([P, S], F16, tag=f"q{m}", name=f"q{m}")
              for m in range(OD // P)]
        kT = [k_pool.tile([P, S], F16, tag=f"k{h}", name=f"k{h}")
              for h in range(NH)]
        for h in range(NH):
            ro = 64 * (h % 2)
            nc.vector.memset(kT[h][64 - ro:128 - ro, :], 0.0)
        v_aug = [v_pool.tile([P, 9, 65], F16, tag=f"v{st}", name=f"v{st}")
                 for st in range(NKC)]
        ones_f = c_pool.tile([P, NH], F32, tag="onef")
        nc.gpsimd.memset(ones_f[:], 1.0)
        ones_r = c_pool.tile([P, NH], F16, tag="oner")
        nc.gpsimd.tensor_copy(ones_r[:], ones_f[:])
        for st in range(NKC):
            nc.gpsimd.memset(v_aug[st][:, 8, :], 0.0)
            nc.gpsimd.tensor_copy(v_aug[st][:, 0:NH, 64:65],
                                  ones_r[:].unsqueeze(2))
        attn = [a_pool.tile([P, S], F16, tag=f"a{m}", name=f"a{m}")
                for m in range(OD // P)]

        def m_sl(i, csl):
            return m_til[i // 2][:, i % 2, csl]

        # ---- K.T projection (PE starts on wk's first half as it lands) ----
        for m in range(OD // P):
            msl = slice(m * P, (m + 1) * P)
            for n in range(2):
                csl = slice(n * 1024, (n + 1) * 1024)
                ps = psum_pool.tile([P, 1024], F32, tag="lg", name="psk")
                for i in range(NDC):
                    for j in range(2):
                        nc.tensor.matmul(
                            ps[:, j * 512:(j + 1) * 512],
                            wk_sb[:, i, msl],
                            m_sl(i, slice(n * 1024 + j * 512,
                                          n * 1024 + (j + 1) * 512)),
                            start=(i == 0), stop=(i == NDC - 1),
                        )
                nc.scalar.activation(
                    kT[2 * m][0:64, csl], ps[0:64, :], AF.Identity,
                    bias=bk_sb[0:64, m:m + 1])
                nc.scalar.activation(
                    kT[2 * m + 1][64:128, csl], ps[64:128, :], AF.Identity,
                    bias=bk_sb[64:128, m:m + 1])

        # ---- V into v_aug ----
        for st in range(NKC):
            ps = psum_pool.tile([P, 1024], F32, tag="lg", name="psv")
            for i in range(NDC):
                nc.tensor.matmul(
                    ps[:, 0:OD], m_sl(i, slice(st * P, (st + 1) * P)),
                    wv_sb[:, i, :],
                    start=(i == 0), stop=(i == NDC - 1),
                )
            nc.vector.tensor_copy(
                v_aug[st][:, 0:NH, 0:64],
                ps[:, 0:OD].rearrange("p (h d) -> p h d", h=NH),
            )

        # ---- x loads: x0 -> wk slot, x2/x3 -> m0/m1 slots ----
        x0_t = w_pool.tile([P, 2, S], F16, tag="wk", name="x0")
        nc.sync.dma_start(out=x0_t[:], in_=x_c[0])
        x2_t = xm_pool.tile([P, 2, S], F16, tag="m0", name="x2")
        nc.sync.dma_start(out=x2_t[:], in_=x_c[2])
        x3_t = xm_pool.tile([P, 2, S], F16, tag="m1", name="x3")
        nc.gpsimd.dma_start(out=x3_t[:], in_=x_c[3])
        x_til = [x0_t, x1_t, x2_t, x3_t]

        def x_sl(i, csl):
            return x_til[i // 2][:, i % 2, csl]

        # ---- Q.T projections (prescaled by SCALE_Q) ----
        for mt in range(OD // P):
            msl = slice(mt * P, (mt + 1) * P)
            for n in range(2):
                csl = slice(n * 1024, (n + 1) * 1024)
                ps = psum_pool.tile([P, 1024], F32, tag="lg", name="psq")
                for i in range(NDC):
                    for j in range(2):
                        nc.tensor.matmul(
                            ps[:, j * 512:(j + 1) * 512],
                            wq_sb[:, i, msl],
                            x_sl(i, slice(n * 1024 + j * 512,
                                          n * 1024 + (j + 1) * 512)),
                            start=(i == 0), stop=(i == NDC - 1),
                        )
                nc.scalar.activation(
                    qT[mt][:, csl], ps[:], AF.Identity,
                    bias=bq_sb[:, mt:mt + 1], scale=SCALE_Q)

        # ---- attention: flat (block, kc) unit stream, pipelined 2 deep
        #      across block boundaries; out-proj after each query-half ----
        blocks = [(mt, 2 * mt + hh, qh)
                  for qh in range(2) for mt in range(4) for hh in range(2)]
        units = [(bi, kc) for bi in range(len(blocks)) for kc in range(NKC)]

        def emit_lg(u):
            bi, kc = units[u]
            mt, h, qh = blocks[bi]
            lg = psum_pool.tile([P, 1024], F32, tag="lg", name="lg")
            for j in range(2):
                nc.tensor.matmul(
                    lg[:, j * 512:(j + 1) * 512],
                    kT[h][:, kc * P:(kc + 1) * P],
                    qT[mt][:, qh * 1024 + j * 512:
                            qh * 1024 + (j + 1) * 512],
                    start=True, stop=True,
                )
            return lg

        def emit_es(u, lg):
            bi, kc = units[u]
            if masked or kc in ACT_KC:
                es = e_pool.tile([P, 1024], F16, tag="es")
                nc.scalar.activation(
                    es[:], lg[:], AF.Exp,
                    bias=mk_sb[:, kc:kc + 1], scale=ACT_SCALE)
                return es[:]
            es = e_pool.tile([P, 1024], I16, tag="es")
            nc.vector._custom_dve(exp_op, out=es[:], in0=lg[:],
                                  s0=EXP_K, s1=EXP_C2, imm2=EXP_IMM2)
            return es[:].bitcast(F16)

        def emit_out_proj(m, qh):
            csl = slice(qh * 1024, (qh + 1) * 1024)
            ps = psum_pool.tile([P, 1024], F32, tag="av", name="pso")
            for i in range(OD // P):
                for j in range(2):
                    nc.tensor.matmul(
                        ps[:, j * 512:(j + 1) * 512],
                        wo_sb[:, i, m * P:(m + 1) * P],
                        attn[i][:, qh * 1024 + j * 512:
                                qh * 1024 + (j + 1) * 512],
                        start=(i == 0), stop=(i == OD // P - 1),
                    )
            ev = o_pool.tile([P, 1024], F16, tag="ev")
            if m % 2 == 0:
                nc.vector.tensor_scalar_add(ev[:], ps[:], bo_sb[:, m:m + 1])
            else:
                nc.scalar.activation(
                    ev[:], ps[:], AF.Identity, bias=bo_sb[:, m:m + 1])
            dq[m % 3].dma_start(out=out_t[m * P:(m + 1) * P, csl], in_=ev[:])

        lg_q = [emit_lg(0), emit_lg(1)]
        av_t = None
        def emit_chain(av, mt, h, qh):
            # softmax denominator: copy (ACT crosses partitions) ->
            # recip -> bcast -> mult
            ro = 64 * (h % 2)
            den = d_pool.tile([1, 1024], F32, tag="den")
            nc.scalar.activation(den[:], av[64:65, :], AF.Copy)
            rcp = d_pool.tile([1, 1024], F32, tag="rcp")
            nc.vector.reciprocal_approx_fast(rcp[:], den[:])
            bc = d_pool.tile([64, 1024], F32, tag="bc")
            nc.gpsimd.partition_broadcast(bc[:], rcp[:])
            nc.vector.tensor_mul(
                attn[mt][ro:ro + 64, qh * 1024:(qh + 1) * 1024],
                av[0:64, :], bc[:])

        for u in range(len(units)):
            bi, kc = units[u]
            mt, h, qh = blocks[bi]
            if kc == 0:
                av_t = psum_pool.tile([P, 1024], F32, tag="av", name="av")
            es_mm = emit_es(u, lg_q.pop(0))
            if u + 2 < len(units):
                lg_q.append(emit_lg(u + 2))
            va_flat = v_aug[kc][:].rearrange("p h d -> p (h d)")
            for j in range(2):
                nc.tensor.matmul(
                    av_t[:, j * 512:(j + 1) * 512],
                    va_flat[:, 65 * h:65 * h + 128],
                    es_mm[:, j * 512:(j + 1) * 512],
                    start=(kc == 0), stop=(kc == NKC - 1),
                )
            if kc == NKC - 1:
                emit_chain(av_t, mt, h, qh)
                if bi % 8 == 7:          # last block of this query-half
                    for m in range(D // P):
                        emit_out_proj(m, qh)

    nc.compile()
    return nc


def _prep_inputs(x, memory, mask, wq, bq, wk, bk, wv, bv, wo, bo):
    f = np.float32
    h = np.float16
    wqT = np.ascontiguousarray(wq.T, dtype=f)
    wkT = np.ascontiguousarray(wk.T, dtype=f)
    wvT = np.ascontiguousarray(wv.T, dtype=f)
    woT = np.ascontiguousarray(wo.T, dtype=f)
    bo_eff = (bo.astype(f) + wo.astype(f) @ bv.astype(f))
    zeros_bo = np.zeros_like(bo_eff)
    in_maps = []
    for c in range(NCORES):
        b, g = divmod(c, 2)
        sl = slice(g * OD, (g + 1) * OD)
        bo_c = bo_eff if g == 0 else zeros_bo
        cst = np.empty((P, 32), f)
        cst[:, 0:4] = bq[sl].astype(f).reshape(OD // P, P).T
        cst[:, 4:8] = bk[sl].astype(f).reshape(OD // P, P).T
        cst[:, 8:16] = bo_c.reshape(D // P, P).T
        cst[:, 16:32] = (
            np.where(mask[b], np.float32(MASK_OCT * ACT_SCALE), 0.0)
            + np.float32(ACT_BIAS)).astype(f).reshape(NKC, P).T
        in_maps.append({
            "x_t": np.ascontiguousarray(x[b].T, dtype=h),
            "mem_t": np.ascontiguousarray(memory[b].T, dtype=h),
            "wq_t": np.ascontiguousarray(wqT[:, sl]).astype(h),
            "wk_t": np.ascontiguousarray(wkT[:, sl]).astype(h),
            "wv_t": np.ascontiguousarray(wvT[:, sl]).astype(h),
            "wo_t": np.ascontiguousarray(woT[sl, :]).astype(h),
            "consts": cst,
        })
    return in_maps


def kernel(x, memory, mask, wq, bq, wk, bk, wv, bv, wo, bo, **run_kwargs):
    x = np.asarray(x, dtype=np.float32)
    memory = np.asarray(memory, dtype=np.float32)
    mask = np.asarray(mask)
    masked = bool(mask.any())
    key = "nc_masked" if masked else "nc"
    if key not in _cache:
        _cache[key] = _build(masked)
    nc = _cache[key]
    in_maps = _prep_inputs(x, memory, mask, wq, bq, wk, bk, wv, bv, wo, bo)
    res = run_bass_kernel_spmd(nc, in_maps, list(range(NCORES)), **run_kwargs)
    out = np.empty((B, S, D), dtype=np.float32)
    for b in range(B):
        part = (res.results[2 * b]["out_t"].astype(np.float32)
                + res.results[2 * b + 1]["out_t"].astype(np.float32))
        out[b] = part.T
    if run_kwargs:
        _cache["last_results"] = res
    return out
